# revision 5
# baseline (speedup 1.0000x reference)
"""B-spline (clamped) surface evaluation on 8 Trainium2 cores.

Math: out[u, v, :] = sum_{a,b} Bu[u,a] * Bv[v,b] * P[su[u]-p+a, sv[v]-p+b, :]

Host precomputes the tiny Cox-de-Boor basis and scatters it into dense
matrices Au [Nu, 64], Av [Nv, 64] so the device work is two dense matmul
stages (TensorEngine friendly, no gathers):

  stage 1:  Tt_d[j, u] = sum_i P[i, j, d] * Au[u, i]          (small matmuls)
  stage 2:  S[u, v, d] = sum_j Tt_d[j, u] * Av[v, j]          (tiled matmuls)

fp32 matmul on TRN2 is ~5x slower than bf16, so operands are split
a = hi + lo (bf16 each) and the K=64 contraction is packed into the full
K=128 PE array: lhsT = [hi; lo] stacked on partitions, one matmul against
[b_hi; b_hi] and one against [b_lo; b_lo] accumulating in PSUM computes the
exact (hi+lo)(b_hi+b_lo) product — 4-pass accuracy for 2 matmuls.

Sharding: data-parallel over u (rows of Au). Each of the 8 cores computes a
[251, 2001, 3] row-slab of the output; host concatenates and trims.
"""

import numpy as np

N_CTRL = 64
N_EVAL = 2001
N_CORES = 8
NU_SHARD = 251  # ceil(2001 / 8); 8 * 251 = 2008 (last 7 rows are zero padding)
U_TILES = [(0, 128), (128, NU_SHARD - 128)]
V_TILE = 512

_CACHE = {}


def _clamped_knots(p, n_ctrl, dtype=np.float64):
    n_internal = n_ctrl - p - 1
    internal = np.linspace(0.0, 1.0, n_internal + 2, dtype=dtype)[1:-1]
    return np.concatenate(
        [np.zeros(p + 1, dtype), internal, np.ones(p + 1, dtype)]
    )


def _dense_basis(params, p, n_ctrl):
    """Dense basis matrix A [len(params), n_ctrl] with A[k, span-p+a] = B[k, a]."""
    knots = _clamped_knots(p, n_ctrl)
    u = np.asarray(params, np.float64)
    spans = np.clip(np.searchsorted(knots, u, side="right") - 1, p, n_ctrl - 1)
    Ns = [np.ones_like(u)]
    left = {}
    right = {}
    for j in range(1, p + 1):
        left[j] = u - knots[spans + 1 - j]
        right[j] = knots[spans + j] - u
        saved = np.zeros_like(u)
        new = []
        for r in range(j):
            temp = Ns[r] / (right[r + 1] + left[j - r])
            new.append(saved + right[r + 1] * temp)
            saved = left[j - r] * temp
        new.append(saved)
        Ns = new
    B = np.stack(Ns, axis=-1)  # [N, p+1]
    A = np.zeros((len(u), n_ctrl), np.float32)
    rows = np.arange(len(u))[:, None]
    cols = spans[:, None] - p + np.arange(p + 1)[None, :]
    A[rows, cols] = B.astype(np.float32)
    return A


def _split_bf16(a):
    """fp32 array -> (hi, lo) bf16 arrays with hi + lo ~= a (~2^-17 rel)."""
    import ml_dtypes

    a = np.ascontiguousarray(a, np.float32)
    hi = a.astype(ml_dtypes.bfloat16)
    lo = (a - hi.astype(np.float32)).astype(ml_dtypes.bfloat16)
    return hi, lo


def _stack_hilo(a):
    """[64, N] fp32 -> [128, N] bf16 with rows 0-63 = hi, 64-127 = lo."""
    hi, lo = _split_bf16(a)
    return np.ascontiguousarray(np.concatenate([hi, lo], axis=0))


def _dup_halves(a_bf16):
    """[64, N] bf16 -> [128, N] with the same data in both partition halves."""
    return np.ascontiguousarray(np.concatenate([a_bf16, a_bf16], axis=0))


def _build_device():
    if "nc" in _CACHE:
        return _CACHE["nc"]

    import concourse.mybir as mybir
    import concourse.tile as tile
    from concourse import bacc

    f32 = mybir.dt.float32
    bf16 = mybir.dt.bfloat16
    nc = bacc.Bacc(
        "TRN2", target_bir_lowering=False, debug=False, num_devices=N_CORES
    )
    in_specs = [
        ("pperm_p", 3 * N_CTRL),  # [pperm_hi; pperm_lo]
        ("aut_hi", NU_SHARD),     # [aut_hi; aut_hi]
        ("aut_lo", NU_SHARD),     # [aut_lo; aut_lo]
        ("avt_hi", N_EVAL),       # [avt_hi; avt_hi]
        ("avt_lo", N_EVAL),       # [avt_lo; avt_lo]
    ]
    ins = {
        name: nc.dram_tensor(name, [128, cols], bf16, kind="ExternalInput").ap()
        for name, cols in in_specs
    }
    out_h = nc.dram_tensor(
        "out", [NU_SHARD, 3 * N_EVAL], f32, kind="ExternalOutput"
    ).ap()

    with tile.TileContext(nc) as tc:
        with (
            tc.tile_pool(name="consts", bufs=1) as consts,
            tc.tile_pool(name="ps1", bufs=2, space="PSUM") as ps1,
            tc.tile_pool(name="ps2", bufs=6, space="PSUM") as ps2,
            tc.tile_pool(name="obuf", bufs=4) as obuf,
        ):
            sb = {}
            for idx, (name, cols) in enumerate(in_specs):
                sb[name] = consts.tile([128, cols], bf16, tag=name, name=name)
                # split input loads across the two HWDGE rings
                eng = nc.sync if idx < 3 else nc.scalar
                eng.dma_start(out=sb[name], in_=ins[name])

            # stage 1: Tt_d[j, u] = sum_i pperm[i, d*64+j] * aut[i, u]
            # K=128 packed: [hi; lo] weights x ([aut_hi; aut_hi] then
            # [aut_lo; aut_lo]) accumulating = exact (hi+lo)(hi+lo) product.
            tt_sb = consts.tile([N_CTRL, 3 * NU_SHARD], f32)
            for d in range(3):
                pt = ps1.tile([N_CTRL, NU_SHARD], f32, tag="pt")
                dsl = slice(d * N_CTRL, (d + 1) * N_CTRL)
                nc.tensor.matmul(
                    pt, sb["pperm_p"][:, dsl], sb["aut_hi"], start=True, stop=False
                )
                nc.tensor.matmul(
                    pt, sb["pperm_p"][:, dsl], sb["aut_lo"], start=False, stop=True
                )
                nc.vector.tensor_copy(
                    tt_sb[:, d * NU_SHARD : (d + 1) * NU_SHARD], pt
                )

            # split Tt (fp32) into bf16 hi/lo, packed as [hi; lo] on partitions
            tt_p = consts.tile([128, 3 * NU_SHARD], bf16)
            tt_hi32 = consts.tile([N_CTRL, 3 * NU_SHARD], f32)
            tt_lo32 = consts.tile([N_CTRL, 3 * NU_SHARD], f32)
            tt_lo = consts.tile([N_CTRL, 3 * NU_SHARD], bf16)
            nc.vector.tensor_copy(tt_p[:N_CTRL], tt_sb)        # hi (bf16 cast)
            nc.vector.tensor_copy(tt_hi32, tt_p[:N_CTRL])      # hi back to fp32
            nc.vector.tensor_sub(tt_lo32, tt_sb, tt_hi32)
            nc.vector.tensor_copy(tt_lo, tt_lo32)
            # lo half lives on partitions 64-127: SBUF->SBUF DMA moves it
            nc.sync.dma_start(out=tt_p[N_CTRL:], in_=tt_lo)

            # stage 2: S[u, v, d] = sum_j Tt_d[j, u] * Av[v, j], interleaved to
            # row layout [u, v*3 + d], DMA'd per (u-tile, v-tile) chunk.
            chunk = 0
            for u0, uw in U_TILES:
                for v0 in range(0, N_EVAL, V_TILE):
                    vw = min(V_TILE, N_EVAL - v0)
                    vsl = slice(v0, v0 + vw)
                    ob = obuf.tile([128, 3 * V_TILE], f32, tag="ob")
                    # all three interleave copies of a chunk go to ONE engine
                    # (they overlap byte-ranges, so Tile serializes them anyway;
                    # alternating whole chunks keeps both engines busy)
                    copy_eng = nc.vector.tensor_copy if chunk % 2 == 0 else nc.scalar.copy
                    for d in range(3):
                        usl = slice(d * NU_SHARD + u0, d * NU_SHARD + u0 + uw)
                        ps = ps2.tile([128, V_TILE], f32, tag="ps")
                        nc.tensor.matmul(
                            ps[:uw, :vw], tt_p[:, usl], sb["avt_hi"][:, vsl],
                            start=True, stop=False,
                        )
                        nc.tensor.matmul(
                            ps[:uw, :vw], tt_p[:, usl], sb["avt_lo"][:, vsl],
                            start=False, stop=True,
                        )
                        copy_eng(ob[:uw, d : 3 * vw : 3], ps[:uw, :vw])
                    # spread output DMA over all three DGE paths
                    dma_eng = (nc.gpsimd, nc.sync, nc.scalar)[chunk % 3]
                    dma_eng.dma_start(
                        out=out_h[u0 : u0 + uw, 3 * v0 : 3 * v0 + 3 * vw],
                        in_=ob[:uw, : 3 * vw],
                    )
                    chunk += 1
    nc.compile()
    _CACHE["nc"] = nc
    return nc


def kernel(control_points, params_u, params_v, degree):
    from concourse.bass_utils import run_bass_kernel_spmd

    p = int(np.asarray(degree))
    cp = np.asarray(control_points, np.float32)
    pu = np.asarray(params_u, np.float32)
    pv = np.asarray(params_v, np.float32)
    assert cp.shape == (N_CTRL, N_CTRL, 3), cp.shape
    assert pu.shape == (N_EVAL,) and pv.shape == (N_EVAL,), (pu.shape, pv.shape)

    Au = np.zeros((N_CORES * NU_SHARD, N_CTRL), np.float32)
    Au[:N_EVAL] = _dense_basis(pu, p, N_CTRL)
    Av = _dense_basis(pv, p, N_CTRL)

    avt_hi, avt_lo = _split_bf16(Av.T)
    avt_hi = _dup_halves(avt_hi)
    avt_lo = _dup_halves(avt_lo)
    pperm_p = _stack_hilo(cp.transpose(0, 2, 1).reshape(N_CTRL, 3 * N_CTRL))

    nc = _build_device()
    in_maps = []
    for c in range(N_CORES):
        aut_hi, aut_lo = _split_bf16(Au[c * NU_SHARD : (c + 1) * NU_SHARD].T)
        in_maps.append(
            {
                "pperm_p": pperm_p,
                "aut_hi": _dup_halves(aut_hi),
                "aut_lo": _dup_halves(aut_lo),
                "avt_hi": avt_hi,
                "avt_lo": avt_lo,
            }
        )

    res = run_bass_kernel_spmd(
        nc,
        in_maps,
        core_ids=list(range(N_CORES)),
        trace=_CACHE.get("trace", False),
        **_CACHE.get("run_kwargs", {}),
    )
    _CACHE["last_result"] = res
    full = np.concatenate([r["out"] for r in res.results], axis=0)[:N_EVAL]
    return np.ascontiguousarray(full.reshape(N_EVAL, N_EVAL, 3))


# revision 7
# speedup vs baseline: 1.2725x; 1.2725x over previous
"""B-spline (clamped) surface evaluation on 8 Trainium2 cores.

Math: out[u, v, :] = sum_{a,b} Bu[u,a] * Bv[v,b] * P[su[u]-p+a, sv[v]-p+b, :]

Host precomputes the tiny Cox-de-Boor basis and scatters it into dense
matrices Au [Nu, 64], Av [Nv, 64] so the device work is two dense matmul
stages (TensorEngine friendly, no gathers):

  stage 1:  Tt_d[j, u] = sum_i P[i, j, d] * Au[u, i]          (small matmuls)
  stage 2:  S[u, v, d] = sum_j Tt_d[j, u] * Av[v, j]          (tiled matmuls)

fp32 matmul on TRN2 is ~5x slower than bf16, so operands are split
a = hi + lo (bf16 each) and the K=64 contraction is packed into the full
K=128 PE array: lhsT = [hi; lo] stacked on partitions, one matmul against
[b_hi; b_hi] and one against [b_lo; b_lo] accumulating in PSUM computes the
exact (hi+lo)(b_hi+b_lo) product — 4-pass accuracy for 2 matmuls.

Sharding: data-parallel over u (rows of Au). Each of the 8 cores computes a
[251, 2001, 3] row-slab of the output; host concatenates and trims.
"""

import numpy as np

N_CTRL = 64
N_EVAL = 2001
N_CORES = 8
NU_SHARD = 251  # ceil(2001 / 8); 8 * 251 = 2008 (last 7 rows are zero padding)
U_TILES = [(0, 128), (128, NU_SHARD - 128)]
V_TILE = 256

_CACHE = {}


def _clamped_knots(p, n_ctrl, dtype=np.float64):
    n_internal = n_ctrl - p - 1
    internal = np.linspace(0.0, 1.0, n_internal + 2, dtype=dtype)[1:-1]
    return np.concatenate(
        [np.zeros(p + 1, dtype), internal, np.ones(p + 1, dtype)]
    )


def _dense_basis(params, p, n_ctrl):
    """Dense basis matrix A [len(params), n_ctrl] with A[k, span-p+a] = B[k, a]."""
    knots = _clamped_knots(p, n_ctrl)
    u = np.asarray(params, np.float64)
    spans = np.clip(np.searchsorted(knots, u, side="right") - 1, p, n_ctrl - 1)
    Ns = [np.ones_like(u)]
    left = {}
    right = {}
    for j in range(1, p + 1):
        left[j] = u - knots[spans + 1 - j]
        right[j] = knots[spans + j] - u
        saved = np.zeros_like(u)
        new = []
        for r in range(j):
            temp = Ns[r] / (right[r + 1] + left[j - r])
            new.append(saved + right[r + 1] * temp)
            saved = left[j - r] * temp
        new.append(saved)
        Ns = new
    B = np.stack(Ns, axis=-1)  # [N, p+1]
    A = np.zeros((len(u), n_ctrl), np.float32)
    rows = np.arange(len(u))[:, None]
    cols = spans[:, None] - p + np.arange(p + 1)[None, :]
    A[rows, cols] = B.astype(np.float32)
    return A


def _split_bf16(a):
    """fp32 array -> (hi, lo) bf16 arrays with hi + lo ~= a (~2^-17 rel)."""
    import ml_dtypes

    a = np.ascontiguousarray(a, np.float32)
    hi = a.astype(ml_dtypes.bfloat16)
    lo = (a - hi.astype(np.float32)).astype(ml_dtypes.bfloat16)
    return hi, lo


def _stack_hilo(a):
    """[64, N] fp32 -> [128, N] bf16 with rows 0-63 = hi, 64-127 = lo."""
    hi, lo = _split_bf16(a)
    return np.ascontiguousarray(np.concatenate([hi, lo], axis=0))


def _dup_halves(a_bf16):
    """[64, N] bf16 -> [128, N] with the same data in both partition halves."""
    return np.ascontiguousarray(np.concatenate([a_bf16, a_bf16], axis=0))


def _build_device():
    if "nc" in _CACHE:
        return _CACHE["nc"]

    import concourse.mybir as mybir
    import concourse.tile as tile
    from concourse import bacc

    f32 = mybir.dt.float32
    bf16 = mybir.dt.bfloat16
    nc = bacc.Bacc(
        "TRN2", target_bir_lowering=False, debug=False, num_devices=N_CORES
    )
    in_specs = [
        ("pperm_p", 3 * N_CTRL),  # [pperm_hi; pperm_lo]
        ("aut_hi", NU_SHARD),     # [aut_hi; aut_hi]
        ("aut_lo", NU_SHARD),     # [aut_lo; aut_lo]
        ("avt_hi", N_EVAL),       # [avt_hi; avt_hi]
        ("avt_lo", N_EVAL),       # [avt_lo; avt_lo]
    ]
    ins = {
        name: nc.dram_tensor(name, [128, cols], bf16, kind="ExternalInput").ap()
        for name, cols in in_specs
    }
    out_h = nc.dram_tensor(
        "out", [NU_SHARD, 3 * N_EVAL], f32, kind="ExternalOutput"
    ).ap()

    with tile.TileContext(nc) as tc:
        with (
            tc.tile_pool(name="consts", bufs=1) as consts,
            tc.tile_pool(name="ps1", bufs=2, space="PSUM") as ps1,
            tc.tile_pool(name="ps2", bufs=6, space="PSUM") as ps2,
            tc.tile_pool(name="obuf", bufs=4) as obuf,
        ):
            sb = {}
            for idx, (name, cols) in enumerate(in_specs):
                sb[name] = consts.tile([128, cols], bf16, tag=name, name=name)
                # split input loads across the two HWDGE rings
                eng = nc.sync if idx < 3 else nc.scalar
                eng.dma_start(out=sb[name], in_=ins[name])

            # stage 1: Tt_d[j, u] = sum_i pperm[i, d*64+j] * aut[i, u]
            # K=128 packed: [hi; lo] weights x ([aut_hi; aut_hi] then
            # [aut_lo; aut_lo]) accumulating = exact (hi+lo)(hi+lo) product.
            tt_sb = consts.tile([N_CTRL, 3 * NU_SHARD], f32)
            for d in range(3):
                pt = ps1.tile([N_CTRL, NU_SHARD], f32, tag="pt")
                dsl = slice(d * N_CTRL, (d + 1) * N_CTRL)
                nc.tensor.matmul(
                    pt, sb["pperm_p"][:, dsl], sb["aut_hi"], start=True, stop=False
                )
                nc.tensor.matmul(
                    pt, sb["pperm_p"][:, dsl], sb["aut_lo"], start=False, stop=True
                )
                nc.vector.tensor_copy(
                    tt_sb[:, d * NU_SHARD : (d + 1) * NU_SHARD], pt
                )

            # split Tt (fp32) into bf16 hi/lo, packed as [hi; lo] on partitions
            tt_p = consts.tile([128, 3 * NU_SHARD], bf16)
            tt_hi32 = consts.tile([N_CTRL, 3 * NU_SHARD], f32)
            tt_lo32 = consts.tile([N_CTRL, 3 * NU_SHARD], f32)
            tt_lo = consts.tile([N_CTRL, 3 * NU_SHARD], bf16)
            nc.vector.tensor_copy(tt_p[:N_CTRL], tt_sb)        # hi (bf16 cast)
            nc.vector.tensor_copy(tt_hi32, tt_p[:N_CTRL])      # hi back to fp32
            nc.vector.tensor_sub(tt_lo32, tt_sb, tt_hi32)
            nc.vector.tensor_copy(tt_lo, tt_lo32)
            # lo half lives on partitions 64-127: SBUF->SBUF DMA moves it
            nc.sync.dma_start(out=tt_p[N_CTRL:], in_=tt_lo)

            # stage 2: S[u, v, d] = sum_j Tt_d[j, u] * Av[v, j], interleaved to
            # row layout [u, v*3 + d], DMA'd per (u-tile, v-tile) chunk.
            chunk = 0
            for u0, uw in U_TILES:
                for v0 in range(0, N_EVAL, V_TILE):
                    vw = min(V_TILE, N_EVAL - v0)
                    vsl = slice(v0, v0 + vw)
                    ob = obuf.tile([128, 3 * V_TILE], f32, tag="ob")
                    # all three interleave copies of a chunk go to ONE engine
                    # (they overlap byte-ranges, so Tile serializes them anyway;
                    # alternating whole chunks keeps both engines busy)
                    copy_eng = nc.vector.tensor_copy if chunk % 2 == 0 else nc.scalar.copy
                    for d in range(3):
                        usl = slice(d * NU_SHARD + u0, d * NU_SHARD + u0 + uw)
                        ps = ps2.tile([128, V_TILE], f32, tag="ps")
                        nc.tensor.matmul(
                            ps[:uw, :vw], tt_p[:, usl], sb["avt_hi"][:, vsl],
                            start=True, stop=False,
                        )
                        nc.tensor.matmul(
                            ps[:uw, :vw], tt_p[:, usl], sb["avt_lo"][:, vsl],
                            start=False, stop=True,
                        )
                        copy_eng(ob[:uw, d : 3 * vw : 3], ps[:uw, :vw])
                    # SWDGE (gpsimd) is the only DGE path that spreads
                    # descriptors over all 16 SDMA engines; HWDGE rings were
                    # observed draining through just 3 engines (~75 GB/s).
                    nc.gpsimd.dma_start(
                        out=out_h[u0 : u0 + uw, 3 * v0 : 3 * v0 + 3 * vw],
                        in_=ob[:uw, : 3 * vw],
                    )
                    chunk += 1
    nc.compile()
    _CACHE["nc"] = nc
    return nc


def kernel(control_points, params_u, params_v, degree):
    from concourse.bass_utils import run_bass_kernel_spmd

    p = int(np.asarray(degree))
    cp = np.asarray(control_points, np.float32)
    pu = np.asarray(params_u, np.float32)
    pv = np.asarray(params_v, np.float32)
    assert cp.shape == (N_CTRL, N_CTRL, 3), cp.shape
    assert pu.shape == (N_EVAL,) and pv.shape == (N_EVAL,), (pu.shape, pv.shape)

    Au = np.zeros((N_CORES * NU_SHARD, N_CTRL), np.float32)
    Au[:N_EVAL] = _dense_basis(pu, p, N_CTRL)
    Av = _dense_basis(pv, p, N_CTRL)

    avt_hi, avt_lo = _split_bf16(Av.T)
    avt_hi = _dup_halves(avt_hi)
    avt_lo = _dup_halves(avt_lo)
    pperm_p = _stack_hilo(cp.transpose(0, 2, 1).reshape(N_CTRL, 3 * N_CTRL))

    nc = _build_device()
    in_maps = []
    for c in range(N_CORES):
        aut_hi, aut_lo = _split_bf16(Au[c * NU_SHARD : (c + 1) * NU_SHARD].T)
        in_maps.append(
            {
                "pperm_p": pperm_p,
                "aut_hi": _dup_halves(aut_hi),
                "aut_lo": _dup_halves(aut_lo),
                "avt_hi": avt_hi,
                "avt_lo": avt_lo,
            }
        )

    res = run_bass_kernel_spmd(
        nc,
        in_maps,
        core_ids=list(range(N_CORES)),
        trace=_CACHE.get("trace", False),
        **_CACHE.get("run_kwargs", {}),
    )
    _CACHE["last_result"] = res
    full = np.concatenate([r["out"] for r in res.results], axis=0)[:N_EVAL]
    return np.ascontiguousarray(full.reshape(N_EVAL, N_EVAL, 3))


# revision 9
# speedup vs baseline: 1.5576x; 1.2240x over previous
"""B-spline (clamped) surface evaluation on 8 Trainium2 cores.

Math: out[u, v, :] = sum_{a,b} Bu[u,a] * Bv[v,b] * P[su[u]-p+a, sv[v]-p+b, :]

Host precomputes the tiny Cox-de-Boor basis and scatters it into dense
matrices Au [Nu, 64], Av [Nv, 64] so the device work is two dense matmul
stages (TensorEngine friendly, no gathers):

  stage 1:  Tt_d[j, u] = sum_i P[i, j, d] * Au[u, i]          (small matmuls)
  stage 2:  S[u, v, d] = sum_j Tt_d[j, u] * Av[v, j]          (tiled matmuls)

fp32 matmul on TRN2 is ~5x slower than bf16, so operands are split
a = hi + lo (bf16 each) and the K=64 contraction is packed into the full
K=128 PE array: lhsT = [hi; lo] stacked on partitions, one matmul against
[b_hi; b_hi] and one against [b_lo; b_lo] accumulating in PSUM computes the
exact (hi+lo)(b_hi+b_lo) product — 4-pass accuracy for 2 matmuls.

Sharding: data-parallel over u (rows of Au). Each of the 8 cores computes a
[251, 2001, 3] row-slab of the output; host concatenates and trims.
"""

import numpy as np

N_CTRL = 64
N_EVAL = 2001
N_CORES = 8
NU_SHARD = 251  # ceil(2001 / 8); 8 * 251 = 2008 (last 7 rows are zero padding)
U_TILES = [(0, 128), (128, NU_SHARD - 128)]
V_TILE = 512

_CACHE = {}


def _clamped_knots(p, n_ctrl, dtype=np.float64):
    n_internal = n_ctrl - p - 1
    internal = np.linspace(0.0, 1.0, n_internal + 2, dtype=dtype)[1:-1]
    return np.concatenate(
        [np.zeros(p + 1, dtype), internal, np.ones(p + 1, dtype)]
    )


def _dense_basis(params, p, n_ctrl):
    """Dense basis matrix A [len(params), n_ctrl] with A[k, span-p+a] = B[k, a]."""
    knots = _clamped_knots(p, n_ctrl)
    u = np.asarray(params, np.float64)
    spans = np.clip(np.searchsorted(knots, u, side="right") - 1, p, n_ctrl - 1)
    Ns = [np.ones_like(u)]
    left = {}
    right = {}
    for j in range(1, p + 1):
        left[j] = u - knots[spans + 1 - j]
        right[j] = knots[spans + j] - u
        saved = np.zeros_like(u)
        new = []
        for r in range(j):
            temp = Ns[r] / (right[r + 1] + left[j - r])
            new.append(saved + right[r + 1] * temp)
            saved = left[j - r] * temp
        new.append(saved)
        Ns = new
    B = np.stack(Ns, axis=-1)  # [N, p+1]
    A = np.zeros((len(u), n_ctrl), np.float32)
    rows = np.arange(len(u))[:, None]
    cols = spans[:, None] - p + np.arange(p + 1)[None, :]
    A[rows, cols] = B.astype(np.float32)
    return A


def _split_bf16(a):
    """fp32 array -> (hi, lo) bf16 arrays with hi + lo ~= a (~2^-17 rel)."""
    import ml_dtypes

    a = np.ascontiguousarray(a, np.float32)
    hi = a.astype(ml_dtypes.bfloat16)
    lo = (a - hi.astype(np.float32)).astype(ml_dtypes.bfloat16)
    return hi, lo


def _stack_hilo(a):
    """[64, N] fp32 -> [128, N] bf16 with rows 0-63 = hi, 64-127 = lo."""
    hi, lo = _split_bf16(a)
    return np.ascontiguousarray(np.concatenate([hi, lo], axis=0))


def _dup_halves(a_bf16):
    """[64, N] bf16 -> [128, N] with the same data in both partition halves."""
    return np.ascontiguousarray(np.concatenate([a_bf16, a_bf16], axis=0))


def _build_device():
    if "nc" in _CACHE:
        return _CACHE["nc"]

    import concourse.mybir as mybir
    import concourse.tile as tile
    from concourse import bacc

    f32 = mybir.dt.float32
    bf16 = mybir.dt.bfloat16
    nc = bacc.Bacc(
        "TRN2", target_bir_lowering=False, debug=False, num_devices=N_CORES
    )
    in_specs = [
        ("pperm_p", 3 * N_CTRL),  # [pperm_hi; pperm_lo]
        ("aut_hi", NU_SHARD),     # [aut_hi; aut_hi]
        ("aut_lo", NU_SHARD),     # [aut_lo; aut_lo]
        ("avt_hi", N_EVAL),       # [avt_hi; avt_hi]
        ("avt_lo", N_EVAL),       # [avt_lo; avt_lo]
    ]
    ins = {
        name: nc.dram_tensor(name, [128, cols], bf16, kind="ExternalInput").ap()
        for name, cols in in_specs
    }
    out_h = nc.dram_tensor(
        "out", [NU_SHARD, 3 * N_EVAL], f32, kind="ExternalOutput"
    ).ap()

    with tile.TileContext(nc) as tc:
        with (
            tc.tile_pool(name="consts", bufs=1) as consts,
            tc.tile_pool(name="ps1", bufs=2, space="PSUM") as ps1,
            tc.tile_pool(name="ps2", bufs=6, space="PSUM") as ps2,
            tc.tile_pool(name="obuf", bufs=6) as obuf,
        ):
            sb = {}
            for idx, (name, cols) in enumerate(in_specs):
                sb[name] = consts.tile([128, cols], bf16, tag=name, name=name)
                # split input loads across the two HWDGE rings
                eng = nc.sync if idx < 3 else nc.scalar
                eng.dma_start(out=sb[name], in_=ins[name])

            # stage 1: Tt_d[j, u] = sum_i pperm[i, d*64+j] * aut[i, u]
            # K=128 packed: [hi; lo] weights x ([aut_hi; aut_hi] then
            # [aut_lo; aut_lo]) accumulating = exact (hi+lo)(hi+lo) product.
            tt_sb = consts.tile([N_CTRL, 3 * NU_SHARD], f32)
            for d in range(3):
                pt = ps1.tile([N_CTRL, NU_SHARD], f32, tag="pt")
                dsl = slice(d * N_CTRL, (d + 1) * N_CTRL)
                nc.tensor.matmul(
                    pt, sb["pperm_p"][:, dsl], sb["aut_hi"], start=True, stop=False
                )
                nc.tensor.matmul(
                    pt, sb["pperm_p"][:, dsl], sb["aut_lo"], start=False, stop=True
                )
                nc.vector.tensor_copy(
                    tt_sb[:, d * NU_SHARD : (d + 1) * NU_SHARD], pt
                )

            # split Tt (fp32) into bf16 hi/lo, packed as [hi; lo] on partitions
            tt_p = consts.tile([128, 3 * NU_SHARD], bf16)
            tt_hi32 = consts.tile([N_CTRL, 3 * NU_SHARD], f32)
            tt_lo32 = consts.tile([N_CTRL, 3 * NU_SHARD], f32)
            tt_lo = consts.tile([N_CTRL, 3 * NU_SHARD], bf16)
            nc.vector.tensor_copy(tt_p[:N_CTRL], tt_sb)        # hi (bf16 cast)
            nc.vector.tensor_copy(tt_hi32, tt_p[:N_CTRL])      # hi back to fp32
            nc.vector.tensor_sub(tt_lo32, tt_sb, tt_hi32)
            nc.vector.tensor_copy(tt_lo, tt_lo32)
            # lo half lives on partitions 64-127: SBUF->SBUF DMA moves it
            nc.sync.dma_start(out=tt_p[N_CTRL:], in_=tt_lo)

            # stage 2: S[u, v, d] = sum_j Tt_d[j, u] * Av[v, j], interleaved to
            # row layout [u, v*3 + d], DMA'd per (u-tile, v-tile) chunk.
            chunk = 0
            for u0, uw in U_TILES:
                for v0 in range(0, N_EVAL, V_TILE):
                    vw = min(V_TILE, N_EVAL - v0)
                    vsl = slice(v0, v0 + vw)
                    ob = obuf.tile([128, 3 * V_TILE], f32, tag="ob")
                    # all three interleave copies of a chunk go to ONE engine
                    # (they overlap byte-ranges, so Tile serializes them anyway;
                    # alternating whole chunks keeps both engines busy)
                    copy_eng = nc.vector.tensor_copy if chunk % 2 == 0 else nc.scalar.copy
                    for d in range(3):
                        usl = slice(d * NU_SHARD + u0, d * NU_SHARD + u0 + uw)
                        ps = ps2.tile([128, V_TILE], f32, tag="ps")
                        nc.tensor.matmul(
                            ps[:uw, :vw], tt_p[:, usl], sb["avt_hi"][:, vsl],
                            start=True, stop=False,
                        )
                        nc.tensor.matmul(
                            ps[:uw, :vw], tt_p[:, usl], sb["avt_lo"][:, vsl],
                            start=False, stop=True,
                        )
                        copy_eng(ob[:uw, d : 3 * vw : 3], ps[:uw, :vw])
                    # SWDGE (gpsimd) is the only DGE path that spreads
                    # descriptors over all 16 SDMA engines; HWDGE rings were
                    # observed draining through just 3 engines (~75 GB/s).
                    nc.gpsimd.dma_start(
                        out=out_h[u0 : u0 + uw, 3 * v0 : 3 * v0 + 3 * vw],
                        in_=ob[:uw, : 3 * vw],
                    )
                    chunk += 1
    nc.compile()
    _CACHE["nc"] = nc
    return nc


def kernel(control_points, params_u, params_v, degree):
    from concourse.bass_utils import run_bass_kernel_spmd

    p = int(np.asarray(degree))
    cp = np.asarray(control_points, np.float32)
    pu = np.asarray(params_u, np.float32)
    pv = np.asarray(params_v, np.float32)
    assert cp.shape == (N_CTRL, N_CTRL, 3), cp.shape
    assert pu.shape == (N_EVAL,) and pv.shape == (N_EVAL,), (pu.shape, pv.shape)

    Au = np.zeros((N_CORES * NU_SHARD, N_CTRL), np.float32)
    Au[:N_EVAL] = _dense_basis(pu, p, N_CTRL)
    Av = _dense_basis(pv, p, N_CTRL)

    avt_hi, avt_lo = _split_bf16(Av.T)
    avt_hi = _dup_halves(avt_hi)
    avt_lo = _dup_halves(avt_lo)
    pperm_p = _stack_hilo(cp.transpose(0, 2, 1).reshape(N_CTRL, 3 * N_CTRL))

    nc = _build_device()
    in_maps = []
    for c in range(N_CORES):
        aut_hi, aut_lo = _split_bf16(Au[c * NU_SHARD : (c + 1) * NU_SHARD].T)
        in_maps.append(
            {
                "pperm_p": pperm_p,
                "aut_hi": _dup_halves(aut_hi),
                "aut_lo": _dup_halves(aut_lo),
                "avt_hi": avt_hi,
                "avt_lo": avt_lo,
            }
        )

    res = run_bass_kernel_spmd(
        nc,
        in_maps,
        core_ids=list(range(N_CORES)),
        trace=_CACHE.get("trace", False),
        **_CACHE.get("run_kwargs", {}),
    )
    _CACHE["last_result"] = res
    full = np.concatenate([r["out"] for r in res.results], axis=0)[:N_EVAL]
    return np.ascontiguousarray(full.reshape(N_EVAL, N_EVAL, 3))


# revision 10
# speedup vs baseline: 1.6116x; 1.0347x over previous
"""B-spline (clamped) surface evaluation on 8 Trainium2 cores.

Math: out[u, v, :] = sum_{a,b} Bu[u,a] * Bv[v,b] * P[su[u]-p+a, sv[v]-p+b, :]

Host precomputes the tiny Cox-de-Boor basis, scatters it into dense matrices
Au [Nu, 64], Av [Nv, 64], and folds the small control-point contraction
T[u, j, d] = sum_i Au[u, i] P[i, j, d] (25M MACs, fp64 on host). The device
then does the dominant contraction (768M MACs, 48 MB output):

  S[u, v, d] = sum_j T[u, j, d] * Av[v, j]       (TensorEngine matmuls)

fp32 matmul on TRN2 is ~5x slower than bf16, so operands are split
a = hi + lo (bf16 each) and the K=64 contraction is packed into the full
K=128 PE array: lhsT = [hi; lo] stacked on partitions; one matmul against
[b_hi; b_hi] plus one against [b_lo; b_lo] accumulating in PSUM computes the
exact (hi+lo)(b_hi+b_lo) product — 4-pass accuracy for 2 matmuls.

The device writes d-plane rows (out[u] = [S(u,:,0) | S(u,:,1) | S(u,:,2)]);
the host interleaves to [Nu, Nv, 3] while unsharding. PSUM->SBUF copies of
different d-planes touch disjoint byte ranges, so Tile runs them on
VectorE/ScalarE concurrently. All DMA goes through the gpsimd SWDGE path —
the HWDGE rings were observed draining through only 3 of 16 SDMA engines.

Sharding: data-parallel over u. Each core computes a [251, 2001, 3] slab.
"""

import numpy as np

N_CTRL = 64
N_EVAL = 2001
N_CORES = 8
NU_SHARD = 251  # ceil(2001 / 8); 8 * 251 = 2008 (last 7 rows are zero padding)
U_TILES = [(0, 128), (128, NU_SHARD - 128)]
V_TILE = 512

_CACHE = {}


def _clamped_knots(p, n_ctrl, dtype=np.float64):
    n_internal = n_ctrl - p - 1
    internal = np.linspace(0.0, 1.0, n_internal + 2, dtype=dtype)[1:-1]
    return np.concatenate(
        [np.zeros(p + 1, dtype), internal, np.ones(p + 1, dtype)]
    )


def _dense_basis(params, p, n_ctrl):
    """Dense basis matrix A [len(params), n_ctrl], float64, with
    A[k, span-p+a] = B[k, a] (Cox-de-Boor, NURBS book A2.2)."""
    knots = _clamped_knots(p, n_ctrl)
    u = np.asarray(params, np.float64)
    spans = np.clip(np.searchsorted(knots, u, side="right") - 1, p, n_ctrl - 1)
    Ns = [np.ones_like(u)]
    left = {}
    right = {}
    for j in range(1, p + 1):
        left[j] = u - knots[spans + 1 - j]
        right[j] = knots[spans + j] - u
        saved = np.zeros_like(u)
        new = []
        for r in range(j):
            temp = Ns[r] / (right[r + 1] + left[j - r])
            new.append(saved + right[r + 1] * temp)
            saved = left[j - r] * temp
        new.append(saved)
        Ns = new
    B = np.stack(Ns, axis=-1)  # [N, p+1]
    A = np.zeros((len(u), n_ctrl), np.float64)
    rows = np.arange(len(u))[:, None]
    cols = spans[:, None] - p + np.arange(p + 1)[None, :]
    A[rows, cols] = B
    return A


def _split_bf16(a):
    """fp32 array -> (hi, lo) bf16 arrays with hi + lo ~= a (~2^-17 rel)."""
    import ml_dtypes

    a = np.ascontiguousarray(a, np.float32)
    hi = a.astype(ml_dtypes.bfloat16)
    lo = (a - hi.astype(np.float32)).astype(ml_dtypes.bfloat16)
    return hi, lo


def _stack_hilo(a):
    """[64, N] fp32 -> [128, N] bf16 with rows 0-63 = hi, 64-127 = lo."""
    hi, lo = _split_bf16(a)
    return np.ascontiguousarray(np.concatenate([hi, lo], axis=0))


def _dup_halves(a_bf16):
    """[64, N] bf16 -> [128, N] with the same data in both partition halves."""
    return np.ascontiguousarray(np.concatenate([a_bf16, a_bf16], axis=0))


def _build_device():
    if "nc" in _CACHE:
        return _CACHE["nc"]

    import concourse.mybir as mybir
    import concourse.tile as tile
    from concourse import bacc

    f32 = mybir.dt.float32
    bf16 = mybir.dt.bfloat16
    nc = bacc.Bacc(
        "TRN2", target_bir_lowering=False, debug=False, num_devices=N_CORES
    )
    in_specs = [
        ("tt_p", 3 * NU_SHARD),   # [T_hi; T_lo], cols d*NU_SHARD + u
        ("avt_hi", N_EVAL),       # [avt_hi; avt_hi]
        ("avt_lo", N_EVAL),       # [avt_lo; avt_lo]
    ]
    ins = {
        name: nc.dram_tensor(name, [128, cols], bf16, kind="ExternalInput").ap()
        for name, cols in in_specs
    }
    # d-plane row layout: row u = [d0 | d1 | d2], each N_EVAL wide
    out_h = nc.dram_tensor(
        "out", [NU_SHARD, 3 * N_EVAL], f32, kind="ExternalOutput"
    ).ap()

    with tile.TileContext(nc) as tc:
        with (
            tc.tile_pool(name="consts", bufs=1) as consts,
            tc.tile_pool(name="ps2", bufs=6, space="PSUM") as ps2,
            tc.tile_pool(name="obuf", bufs=3) as obuf,
        ):
            sb = {}
            for name, cols in in_specs:
                sb[name] = consts.tile([128, cols], bf16, tag=name, name=name)
                nc.gpsimd.dma_start(out=sb[name], in_=ins[name])

            # S[u, v, d] = sum_j Tt_d[j, u] * Av[v, j]
            n_copy = 0
            for u0, uw in U_TILES:
                for d in range(3):
                    usl = slice(d * NU_SHARD + u0, d * NU_SHARD + u0 + uw)
                    ob = obuf.tile([128, N_EVAL], f32, tag="ob")
                    for v0 in range(0, N_EVAL, V_TILE):
                        vw = min(V_TILE, N_EVAL - v0)
                        vsl = slice(v0, v0 + vw)
                        ps = ps2.tile([128, V_TILE], f32, tag="ps")
                        nc.tensor.matmul(
                            ps[:uw, :vw], sb["tt_p"][:, usl],
                            sb["avt_hi"][:, vsl], start=True, stop=False,
                        )
                        nc.tensor.matmul(
                            ps[:uw, :vw], sb["tt_p"][:, usl],
                            sb["avt_lo"][:, vsl], start=False, stop=True,
                        )
                        # alternate engines; different (d, vt) regions are
                        # disjoint, so DVE and ACT copies run concurrently
                        if n_copy % 2 == 0:
                            nc.vector.tensor_copy(ob[:uw, vsl], ps[:uw, :vw])
                        else:
                            nc.scalar.copy(ob[:uw, vsl], ps[:uw, :vw])
                        n_copy += 1
                    nc.gpsimd.dma_start(
                        out=out_h[u0 : u0 + uw, d * N_EVAL : (d + 1) * N_EVAL],
                        in_=ob[:uw],
                    )
    nc.compile()
    _CACHE["nc"] = nc
    return nc


def kernel(control_points, params_u, params_v, degree):
    from concourse.bass_utils import run_bass_kernel_spmd

    p = int(np.asarray(degree))
    cp = np.asarray(control_points, np.float32)
    pu = np.asarray(params_u, np.float32)
    pv = np.asarray(params_v, np.float32)
    assert cp.shape == (N_CTRL, N_CTRL, 3), cp.shape
    assert pu.shape == (N_EVAL,) and pv.shape == (N_EVAL,), (pu.shape, pv.shape)

    Au = np.zeros((N_CORES * NU_SHARD, N_CTRL), np.float64)
    Au[:N_EVAL] = _dense_basis(pu, p, N_CTRL)
    Av = _dense_basis(pv, p, N_CTRL)

    # host stage 1 (0.3% of the FLOPs): T[j, d, u] = sum_i P[i,j,d] Au[u,i]
    T = (cp.astype(np.float64).transpose(1, 2, 0).reshape(3 * N_CTRL, N_CTRL)
         @ Au.T).reshape(N_CTRL, 3, N_CORES * NU_SHARD)

    avt_hi, avt_lo = _split_bf16(Av.T.astype(np.float32))
    avt_hi = _dup_halves(avt_hi)
    avt_lo = _dup_halves(avt_lo)

    nc = _build_device()
    in_maps = []
    for c in range(N_CORES):
        tt = T[:, :, c * NU_SHARD : (c + 1) * NU_SHARD].reshape(N_CTRL, -1)
        in_maps.append(
            {
                "tt_p": _stack_hilo(tt.astype(np.float32)),
                "avt_hi": avt_hi,
                "avt_lo": avt_lo,
            }
        )

    res = run_bass_kernel_spmd(
        nc,
        in_maps,
        core_ids=list(range(N_CORES)),
        trace=_CACHE.get("trace", False),
        **_CACHE.get("run_kwargs", {}),
    )
    _CACHE["last_result"] = res
    full = np.concatenate([r["out"] for r in res.results], axis=0)[:N_EVAL]
    # d-plane rows -> [Nu, Nv, 3]
    return np.ascontiguousarray(
        full.reshape(N_EVAL, 3, N_EVAL).transpose(0, 2, 1)
    )
